# revision 5
# baseline (speedup 1.0000x reference)
"""Multi-head attention block (pre-LN, residual) on 8 Trainium2 NeuronCores.

Sharding: (batch x head-group) grid. Core c handles batch b = c//2 and head
group g = c%2 (8 of 16 heads). Per core: LN(x_b) -> per-head QKV projections
-> softmax attention (no max-subtraction; scores are O(10)) -> out-projection
against the local 512-wide slice of Wo, + 0.5*(x+bo) residual. Host sums the
two partial outputs per batch (the pair all-reduce) and stacks batches.

LayerNorm gamma/beta are folded into the QKV weights/biases on the host
(exact: projections are linear in xn). The 1/sqrt(E) score scale is folded
into Wq. Matmul operands are bf16 with fp32 PSUM accumulation; LN statistics,
softmax denominators and the residual path stay fp32.
"""

import numpy as np
import ml_dtypes

import concourse.bass as bass
import concourse.mybir as mybir
import concourse.tile as tile
from concourse import bacc
from concourse import bass_utils
from concourse.bass import ts
from concourse.masks import make_identity

BF_NP = ml_dtypes.bfloat16

B, S, D = 4, 2048, 1024
H, E = 16, 64
LN_EPS = 1e-5
SCALE = 8.0  # sqrt(E) * TEMP

N_CORES = 8
HL = H // 2          # heads per core
ST = S // 128        # 16 s-tiles of 128
KT = D // 128        # 8 contraction tiles for D
NP_ = HL // 2        # 4 head pairs per core
NB = S // 512        # 4 s-blocks of 512
TT = S // 128        # 16 t-tiles of 128

F32 = mybir.dt.float32
BF = mybir.dt.bfloat16

_NC_CACHE = None


def _emit(nc, aps):
    x_ap = aps["x"]
    xr_ap = aps["xr"]
    wq_ap, wk_ap, wv_ap, wo_ap = aps["wq"], aps["wk"], aps["wv"], aps["wo"]
    bq_ap, bk_ap, bv_ap = aps["bq"], aps["bk"], aps["bv"]
    out_ap = aps["out"]

    tc = aps["tc"]
    import contextlib

    ctx = contextlib.ExitStack()
    with ctx:
        const = ctx.enter_context(tc.tile_pool(name="const", bufs=1))
        big = ctx.enter_context(tc.tile_pool(name="big", bufs=1))
        xin = ctx.enter_context(tc.tile_pool(name="xin", bufs=3))
        stat = ctx.enter_context(tc.tile_pool(name="stat", bufs=4))
        xnp = ctx.enter_context(tc.tile_pool(name="xnp", bufs=3))
        ptp = ctx.enter_context(tc.tile_pool(name="ptp", bufs=3))
        denp = ctx.enter_context(tc.tile_pool(name="denp", bufs=2))
        rdenp = ctx.enter_context(tc.tile_pool(name="rdenp", bufs=2))
        xrp = ctx.enter_context(tc.tile_pool(name="xrp", bufs=2))
        outp = ctx.enter_context(tc.tile_pool(name="outp", bufs=3))
        psA = ctx.enter_context(tc.tile_pool(name="psA", bufs=6, space="PSUM"))
        psB = ctx.enter_context(tc.tile_pool(name="psB", bufs=2, space="PSUM"))

        # ---- constants / weights resident in SBUF ----
        wq_sb = const.tile([128, KT, 512], BF, tag="wq")
        wk_sb = const.tile([128, KT, 512], BF, tag="wk")
        wv_sb = const.tile([128, KT, 512], BF, tag="wv")
        for k in range(KT):
            nc.sync.dma_start(out=wq_sb[:, k, :], in_=wq_ap[k])
            nc.sync.dma_start(out=wk_sb[:, k, :], in_=wk_ap[k])
            nc.sync.dma_start(out=wv_sb[:, k, :], in_=wv_ap[k])
        wo_sb = const.tile([128, 4, 1024], BF, tag="wo")
        for k in range(4):
            nc.sync.dma_start(out=wo_sb[:, k, :], in_=wo_ap[k])
        bq_sb = const.tile([128, NP_], F32, tag="bq")
        bk_sb = const.tile([128, NP_], F32, tag="bk")
        nc.sync.dma_start(out=bq_sb, in_=bq_ap)
        nc.sync.dma_start(out=bk_sb, in_=bk_ap)
        bv_sb = const.tile([128, 512], F32, tag="bv")
        bv_bcast = bass.AP(
            tensor=bv_ap.tensor,
            offset=bv_ap.offset,
            ap=[[0, 128], [1, 512]],
        )
        nc.gpsimd.dma_start(out=bv_sb, in_=bv_bcast)
        ident = const.tile([128, 128], BF, tag="id")
        make_identity(nc, ident)
        ones64 = const.tile([1, 64], F32, tag="o64")
        nc.vector.memset(ones64, 1.0)
        onescol = const.tile([128, 1], BF, tag="oc")
        nc.vector.memset(onescol, 1.0)
        eps_t = const.tile([128, 1], F32, tag="eps")
        nc.vector.memset(eps_t, LN_EPS)

        xnT = big.tile([128, KT, S], BF, tag="xnT")     # [d, s] transposed LN(x)
        qT = big.tile([128, NP_, S], BF, tag="qT")      # [(pairhead,e), s]
        kT_ = big.tile([128, NP_, S], BF, tag="kT")
        v_sb = big.tile([128, TT, 512], BF, tag="v")    # [t, (h,e)]
        hT = big.tile([128, 4, S], BF, tag="hT")        # [(h,e), s] attn output

        # ---- phase 1: LayerNorm + transpose ----
        for i in range(ST):
            x_t = xin.tile([128, D], F32, tag="x")
            nc.sync.dma_start(out=x_t, in_=x_ap[ts(i, 128), :])
            stats = stat.tile([128, 2, 6], F32, tag="st")
            for sg in range(2):
                nc.vector.bn_stats(out=stats[:, sg, :], in_=x_t[:, ts(sg, 512)])
            mv = stat.tile([128, 2], F32, tag="mv")
            nc.vector.bn_aggr(out=mv, in_=stats)
            std = stat.tile([128, 1], F32, tag="sd")
            nc.scalar.activation(
                out=std, in_=mv[:, 1:2],
                func=mybir.ActivationFunctionType.Sqrt, bias=eps_t,
            )
            istd = stat.tile([128, 1], F32, tag="is")
            nc.vector.reciprocal(out=istd, in_=std)
            xn_t = xnp.tile([128, D], BF, tag="xn")
            nc.vector.tensor_scalar(
                out=xn_t, in0=x_t,
                scalar1=mv[:, 0:1], scalar2=istd,
                op0=mybir.AluOpType.subtract, op1=mybir.AluOpType.mult,
            )
            for k in range(KT):
                ps_tr = psA.tile([128, 128], BF, tag="ps")
                nc.tensor.transpose(out=ps_tr, in_=xn_t[:, ts(k, 128)], identity=ident)
                nc.vector.tensor_copy(out=xnT[:, k, ts(i, 128)], in_=ps_tr)

        # ---- phase 2: QKV projections ----
        for w_sb, b_sb, dst in ((wq_sb, bq_sb, qT), (wk_sb, bk_sb, kT_)):
            for p in range(NP_):
                for n in range(NB):
                    ps = psA.tile([128, 512], F32, tag="ps")
                    for k in range(KT):
                        nc.tensor.matmul(
                            ps, lhsT=w_sb[:, k, ts(p, 128)], rhs=xnT[:, k, ts(n, 512)],
                            start=(k == 0), stop=(k == KT - 1),
                        )
                    nc.vector.tensor_scalar_add(
                        out=dst[:, p, ts(n, 512)], in0=ps, scalar1=b_sb[:, p:p + 1]
                    )
        for t in range(TT):
            ps = psA.tile([128, 512], F32, tag="ps")
            for k in range(KT):
                nc.tensor.matmul(
                    ps, lhsT=xnT[:, k, ts(t, 128)], rhs=wv_sb[:, k, :],
                    start=(k == 0), stop=(k == KT - 1),
                )
            nc.vector.tensor_add(out=v_sb[:, t, :], in0=ps, in1=bv_sb)

        # ---- phase 3: attention (scores^T -> exp -> PV), per head-pair/s-block --
        for p in range(NP_):
            for n in range(NB):
                den = denp.tile([128, 2, 512], BF, tag="den")
                pvps = psB.tile([128, 512], F32, tag="pv")
                for t in range(TT):
                    s1 = psA.tile([128, 512], F32, tag="ps")
                    s2 = psA.tile([128, 512], F32, tag="ps")
                    nc.tensor.matmul(
                        s1, lhsT=kT_[0:64, p, ts(t, 128)], rhs=qT[0:64, p, ts(n, 512)],
                        start=True, stop=True, tile_position=(0, 0),
                    )
                    nc.tensor.matmul(
                        s2, lhsT=kT_[64:128, p, ts(t, 128)], rhs=qT[64:128, p, ts(n, 512)],
                        start=True, stop=True, tile_position=(64, 0),
                    )
                    pt1 = ptp.tile([128, 512], BF, tag="pt1")
                    pt2 = ptp.tile([128, 512], BF, tag="pt2")
                    nc.scalar.activation(out=pt1, in_=s1, func=mybir.ActivationFunctionType.Exp)
                    nc.scalar.activation(out=pt2, in_=s2, func=mybir.ActivationFunctionType.Exp)
                    if t == 0:
                        nc.vector.tensor_copy(out=den[:, 0, :], in_=pt1)
                        nc.vector.tensor_copy(out=den[:, 1, :], in_=pt2)
                    else:
                        nc.vector.tensor_add(out=den[:, 0, :], in0=den[:, 0, :], in1=pt1)
                        nc.vector.tensor_add(out=den[:, 1, :], in0=den[:, 1, :], in1=pt2)
                    nc.tensor.matmul(
                        pvps[0:64, :], lhsT=v_sb[:, t, p * 128:p * 128 + 64], rhs=pt1,
                        start=(t == 0), stop=(t == TT - 1), tile_position=(0, 0),
                        skip_group_check=True,
                    )
                    nc.tensor.matmul(
                        pvps[64:128, :], lhsT=v_sb[:, t, p * 128 + 64:p * 128 + 128], rhs=pt2,
                        start=(t == 0), stop=(t == TT - 1), tile_position=(0, 64),
                        skip_group_check=True,
                    )
                # denominators: fp32 partition-reduce via ones matmul
                ps_d1 = psA.tile([1, 512], F32, tag="ps")
                ps_d2 = psA.tile([1, 512], F32, tag="ps")
                nc.tensor.matmul(
                    ps_d1, lhsT=onescol, rhs=den[:, 0, :],
                    start=True, stop=True, skip_group_check=True,
                )
                nc.tensor.matmul(
                    ps_d2, lhsT=onescol, rhs=den[:, 1, :],
                    start=True, stop=True, skip_group_check=True,
                )
                recip = rdenp.tile([1, 1024], F32, tag="rd")
                nc.vector.reciprocal(out=recip[0:1, 0:512], in_=ps_d1)
                nc.vector.reciprocal(out=recip[0:1, 512:1024], in_=ps_d2)
                ps_db = psA.tile([128, 512], F32, tag="ps")
                nc.tensor.matmul(
                    ps_db[0:64, :], lhsT=ones64, rhs=recip[0:1, 0:512],
                    start=True, stop=True, tile_position=(0, 0), skip_group_check=True,
                )
                nc.tensor.matmul(
                    ps_db[64:128, :], lhsT=ones64, rhs=recip[0:1, 512:1024],
                    start=True, stop=True, tile_position=(0, 64), skip_group_check=True,
                )
                db_sb = rdenp.tile([128, 512], F32, tag="db")
                nc.vector.tensor_copy(out=db_sb, in_=ps_db)
                nc.vector.tensor_mul(out=hT[:, p, ts(n, 512)], in0=pvps, in1=db_sb)

        # ---- phase 4: out projection + residual ----
        for i in range(ST):
            xr_t = xrp.tile([128, D], F32, tag="xr")
            nc.sync.dma_start(out=xr_t, in_=xr_ap[ts(i, 128), :])
            for c in range(2):
                ps_o = psA.tile([128, 512], F32, tag="ps")
                for k in range(4):
                    nc.tensor.matmul(
                        ps_o, lhsT=hT[:, k, ts(i, 128)], rhs=wo_sb[:, k, ts(c, 512)],
                        start=(k == 0), stop=(k == 3),
                    )
                osb = outp.tile([128, 512], F32, tag="ob")
                nc.vector.tensor_add(out=osb, in0=ps_o, in1=xr_t[:, ts(c, 512)])
                nc.sync.dma_start(out=out_ap[ts(i, 128), ts(c, 512)], in_=osb)


def build():
    nc = bacc.Bacc("TRN2", target_bir_lowering=False, debug=False, num_devices=N_CORES)
    aps = {
        "x": nc.dram_tensor("x", [S, D], F32, kind="ExternalInput").ap(),
        "xr": nc.dram_tensor("xr", [S, D], F32, kind="ExternalInput").ap(),
        "wq": nc.dram_tensor("wq", [KT, 128, 512], BF, kind="ExternalInput").ap(),
        "wk": nc.dram_tensor("wk", [KT, 128, 512], BF, kind="ExternalInput").ap(),
        "wv": nc.dram_tensor("wv", [KT, 128, 512], BF, kind="ExternalInput").ap(),
        "wo": nc.dram_tensor("wo", [4, 128, 1024], BF, kind="ExternalInput").ap(),
        "bq": nc.dram_tensor("bq", [128, NP_], F32, kind="ExternalInput").ap(),
        "bk": nc.dram_tensor("bk", [128, NP_], F32, kind="ExternalInput").ap(),
        "bv": nc.dram_tensor("bv", [512], F32, kind="ExternalInput").ap(),
        "out": nc.dram_tensor("out", [S, D], F32, kind="ExternalOutput").ap(),
    }
    with tile.TileContext(nc) as tc:
        aps["tc"] = tc
        _emit(nc, aps)
    nc.compile()
    return nc


def prep_core_inputs(x, Wq, bq, Wk, bk, Wv, bv, Wo, bo, ln_gamma, ln_beta):
    """Host-side sharding: returns list of 8 in_maps (numpy arrays)."""
    x = np.asarray(x, np.float32)
    Wq, bq = np.asarray(Wq, np.float32), np.asarray(bq, np.float32)
    Wk, bk = np.asarray(Wk, np.float32), np.asarray(bk, np.float32)
    Wv, bv = np.asarray(Wv, np.float32), np.asarray(bv, np.float32)
    Wo, bo = np.asarray(Wo, np.float32), np.asarray(bo, np.float32)
    gamma, beta = np.asarray(ln_gamma, np.float32), np.asarray(ln_beta, np.float32)

    Wq_eff = Wq * gamma[None, None, :] / SCALE
    bq_eff = (bq + Wq @ beta) / SCALE
    Wk_eff = Wk * gamma[None, None, :]
    bk_eff = bk + Wk @ beta
    Wv_eff = Wv * gamma[None, None, :]
    bv_eff = bv + Wv @ beta

    def wq_layout(w):  # [8, 64, 1024] -> [KT, 128, 512]
        # w[h, e, kt*128+dd] -> out[kt, dd, h*64+e]
        return np.ascontiguousarray(
            w.reshape(HL * E, KT, 128).transpose(1, 2, 0)
        ).astype(BF_NP)

    def b_layout(b):  # [8, 64] -> [128, 4]: out[(hh*64+e), p] = b[2p+hh, e]
        return np.ascontiguousarray(
            b.reshape(NP_, 2 * E).T
        ).astype(np.float32)

    in_maps = []
    for c in range(N_CORES):
        bidx, g = c // 2, c % 2
        hs = slice(g * HL, (g + 1) * HL)
        wo_loc = Wo[:, g * 512:(g + 1) * 512]  # [1024, 512]
        wo_dev = np.ascontiguousarray(
            wo_loc.T.reshape(4, 128, 1024)
        ).astype(BF_NP)
        in_maps.append({
            "x": x[bidx],
            "xr": 0.5 * (x[bidx] + bo[None, :]),
            "wq": wq_layout(Wq_eff[hs]),
            "wk": wq_layout(Wk_eff[hs]),
            "wv": wq_layout(Wv_eff[hs]),
            "wo": wo_dev,
            "bq": b_layout(bq_eff[hs]),
            "bk": b_layout(bk_eff[hs]),
            "bv": bv_eff[hs].reshape(512).astype(np.float32),
            "out": np.zeros((S, D), np.float32),
        })
    return in_maps


def kernel(x, Wq, bq, Wk, bk, Wv, bv, Wo, bo, ln_gamma, ln_beta):
    global _NC_CACHE
    if _NC_CACHE is None:
        _NC_CACHE = build()
    nc = _NC_CACHE
    in_maps = prep_core_inputs(x, Wq, bq, Wk, bk, Wv, bv, Wo, bo, ln_gamma, ln_beta)
    for m in in_maps:
        m.pop("out")
    res = bass_utils.run_bass_kernel_spmd(nc, in_maps, core_ids=list(range(N_CORES)))
    out = np.empty((B, S, D), np.float32)
    for bidx in range(B):
        out[bidx] = res.results[2 * bidx]["out"] + res.results[2 * bidx + 1]["out"]
    return out
